# revision 30
# baseline (speedup 1.0000x reference)
"""Trainium2 Bass kernel: offset-attention transformer block (nn_OA_2b).

Computation (per batch b):
    x_q = (Wq @ q)^T            [N, 64]
    x_k = Wk @ x                [64, N]
    x_v = Wv @ q + bv           [256, N]
    E   = x_q @ x_k             [N, N]
    A   = softmax_rows(E)
    A   = A / (1e-9 + colsum(A))
    x_r = x_v @ A               [256, N]
    t   = Wt @ (x - x_r) + bt
    out = relu(batchnorm(t))    (batch stats over all B and N)

Sharding: data-parallel over batch, one batch per NeuronCore (B=8, 8 cores).
The BatchNorm statistics couple the batches -> one tiny AllGather + local
sum of per-channel (sum, sumsq) at the end.  bt cancels inside the
normalization (it shifts t and mean equally), so it is dropped entirely.

Kernel strategy per core: the N query rows are processed in K=8 slices of
512 rows, software-pipelined so the ACT-bound softmax exp of slice s+1 runs
concurrently with the PE-bound attention-apply matmuls of slice s:
  - inputs are DMAed in 1024-column strips (issued from the idle Pool
    sequencer) and the projections + first slice's energy/exp cascade
    behind the DMA at strip granularity.
  - pass1(s): E strips [128, 1024] in PSUM (bf16 x_q/x_k: the ~0.03 logit
    rounding is well inside the 2e-2 budget and halves the fp32 LDW cost
    and PE power), ACT Exp with accum_out -> U half-chunk tiles (bf16,
    SBUF, 20 rotating slots = 2.5 slices deep so the exp-vs-apply WAR
    lands two stages back; 4 slots reuse the dead q-strip SBUF, 8 are
    later recycled for the t tensor) + row sums.  No max subtraction
    needed: logits are O(40) < 88, exp fits fp32 with bias -40.
  - 1/rowsum is folded into a per-slice scaled bf16 copy of x_v^T (whose
    extra ones-column turns into 1/rs, making the colsum come out of the
    same matmul).
  - pass2(s): x_rT partial [m128, 258] = sum_{n in s} U^T-chunk . xvts-chunk
    accumulated in PSUM over the slice (accp 4 banks deep so the DVE
    drain never stalls the PE), then DVE-added into a persistent SBUF
    accumulator xracc.  PSUM budget: cascade psE4+psB2; slices 0-6
    psE4+accp4; slice 7 swaps psE for accp2(4)+tpp(2).
  - final: scale by 1/(1e-9+colsum) into bf16 (ACT), PE-transpose 128x128
    bf16 blocks, subtract in place into x (producing d = x - x_r).
  - t = WtT^T @ d (no bias); per-strip DVE copy-with-accum gives the
    channel sums and ACT Square-with-accum the sumsq; out = relu(a*t+b)
    with the BN affine folded in, relu split ACT/DVE and stores split
    across two queues.
  - collectives: the 8 cores launch with a persistent ~11us stagger, so
    any single collective pays stagger + cold-ring latency (~30us) while
    a second one right after costs ~7us.  Tiny warm-keeper AllGathers
    tied to slices 1/3/5/6 absorb the stagger *during* compute, the BN
    stats ship in a single gather at the end (splitting it, or adding a
    slice-7 keeper, only serializes more ops and measured slower).
"""

import numpy as np

import concourse.bass as bass
import concourse.bacc as bacc
import concourse.mybir as mybir
import concourse.tile as tile
from concourse.masks import make_identity

F32 = mybir.dt.float32
F32R = mybir.dt.float32r
BF16 = mybir.dt.bfloat16
AF = mybir.ActivationFunctionType
ALU = mybir.AluOpType
AX = mybir.AxisListType

B, CM, DX, N = 8, 256, 128, 4096
CM4 = CM // 4            # 64
NCH = N // 128           # 32 chunks of 128 points
NMG = N // 512           # 8 m-groups of 512
NST = N // 1024          # 4 strips of 1024
BN_EPS = 1e-5
ESHIFT = 40.0          # exp(e - ESHIFT): keeps rowsums well inside range
NCORES = 8
CMP = CM + 2           # augmented width: 256 ch + colsum-ones col + pad (even)
K = 8                  # row slices
CPS = NCH // K         # 4 chunks of 128 rows per slice


def build_nc():
    nc = bacc.Bacc(None, num_devices=NCORES)

    dq = nc.dram_tensor("q", [DX, N], F32R, kind="ExternalInput")
    dx = nc.dram_tensor("x", [CM, N], F32R, kind="ExternalInput")
    dwqT = nc.dram_tensor("wqT", [DX, CM4], F32R, kind="ExternalInput")
    dwkT = nc.dram_tensor("wkT", [CM, CM4], F32R, kind="ExternalInput")
    dwvT = nc.dram_tensor("wvT", [DX, CMP], F32R, kind="ExternalInput")
    dwtT = nc.dram_tensor("wtT", [CM, CM], F32R, kind="ExternalInput")
    dbvb = nc.dram_tensor("bvb", [128, CMP], F32, kind="ExternalInput")
    dga = nc.dram_tensor("gac", [128, 2], F32, kind="ExternalInput")
    dbe = nc.dram_tensor("bec", [128, 2], F32, kind="ExternalInput")
    dout = nc.dram_tensor("out", [CM, N], F32, kind="ExternalOutput")

    with tile.TileContext(nc) as tc:
        _build(nc, tc, dq, dx, dwqT, dwkT, dwvT, dwtT, dbvb, dga, dbe, dout)
    nc.compile()
    return nc


def _build(nc, tc, dq, dx, dwqT, dwkT, dwvT, dwtT, dbvb, dga, dbe, dout):
    from contextlib import ExitStack

    ctx = ExitStack()
    with ctx:
        consts = ctx.enter_context(tc.tile_pool(name="consts", bufs=1))
        pbig = ctx.enter_context(tc.tile_pool(name="pbig", bufs=1))
        small = ctx.enter_context(tc.tile_pool(name="small", bufs=4))

        # ---- constant / weight loads (SP queue) ----
        wqT = consts.tile([128, CM4], F32R)
        nc.sync.dma_start(wqT, dwqT[:])
        wvT = consts.tile([128, CMP], F32R)
        nc.sync.dma_start(wvT, dwvT[:])
        wkT = [consts.tile([128, CM4], F32R, tag=f"wkT{k}", name=f"wkT{k}") for k in range(2)]
        for k in range(2):
            nc.sync.dma_start(wkT[k], dwkT[k * 128:(k + 1) * 128, :])
        bvb = consts.tile([128, CMP], F32)
        nc.sync.dma_start(bvb, dbvb[:])
        wtT = [consts.tile([128, CM], F32R, tag=f"wtT{k}", name=f"wtT{k}") for k in range(2)]
        for k in range(2):
            nc.sync.dma_start(wtT[k], dwtT[k * 128:(k + 1) * 128, :])
        gac = consts.tile([128, 2], F32)
        nc.sync.dma_start(gac, dga[:])
        bec = consts.tile([128, 2], F32)
        nc.sync.dma_start(bec, dbe[:])
        ident = consts.tile([128, 128], F32)
        make_identity(nc, ident[:])
        identb = consts.tile([128, 128], BF16)
        make_identity(nc, identb[:])

        nshift = consts.tile([128, 1], F32)
        nc.vector.memset(nshift, -ESHIFT)


        # ---- input strips (Pool queue: near-zero dispatch cost) ----
        qs = [pbig.tile([128, 1024], F32R, tag=f"q{i}", name=f"qs{i}")
              for i in range(NST)]
        xs = [[pbig.tile([128, 1024], F32R, tag=f"x{c}_{i}", name=f"xs{c}_{i}")
               for i in range(NST)] for c in range(2)]
        nc.gpsimd.dma_start(qs[0], dq[:, 0:1024])
        for i in range(NST):
            for c in range(2):
                nc.gpsimd.dma_start(xs[c][i], dx[c * 128:(c + 1) * 128,
                                                 i * 1024:(i + 1) * 1024])
            if i + 1 < NST:
                nc.gpsimd.dma_start(qs[i + 1], dq[:, (i + 1) * 1024:(i + 2) * 1024])

        # ---- persistent attention tensors ----
        xqT = [pbig.tile([CM4, 1024], BF16, tag=f"xqt{i}", name=f"xqT{i}")
               for i in range(NST)]
        xk = [pbig.tile([CM4, 1024], BF16, tag=f"xk{i}", name=f"xk{i}")
              for i in range(NST)]
        xvt = [pbig.tile([128, CMP], BF16, tag=f"xv{ni}", name=f"xvt{ni}")
               for ni in range(NCH)]
        xracc = [pbig.tile([128, CMP], F32, tag=f"xr{j}", name=f"xr{j}")
                 for j in range(NCH)]                      # persistent x_rT accum

        # ---- PSUM pools: cascade psE 4 + psB 2; slices 0-6 psE 4 +
        # accp 4; slice 7 accp2 4 + tpp 2 (psE closed) = always <= 8 banks
        warm_ctx = ExitStack()
        warmp = warm_ctx.enter_context(
            tc.tile_pool(name="warmp", bufs=1, space="DRAM"))
        actx = ExitStack()
        xvsp = actx.enter_context(tc.tile_pool(name="xvsp", bufs=2))
        rsp_p = actx.enter_context(tc.tile_pool(name="rsp", bufs=2))
        psE_ctx = ExitStack()
        psE = psE_ctx.enter_context(tc.tile_pool(name="psE", bufs=2, space="PSUM"))

        # U = exp(E-40) as 20 rotating [128,2048] bf16 half-chunk tiles
        # (2.5 slices deep -> the exp-vs-apply WAR lands 2 stages back,
        # killing the stage-boundary bubbles). Slots 16-19 reuse the dead
        # q-strip slots (their chunks run from stage 2 on, q dies at ~20us).
        NUH = 20
        uh = [pbig.tile([128, 2048], BF16,
                        tag=(f"q{k - 16}" if k >= 16 else f"uh{k}"),
                        name=f"uhv2_{k}")
              for k in range(NUH)]

        def u_ap(ng, half):
            return uh[(2 * ng + half) % NUH]

        cascade_ctx = ExitStack()
        xvts = [None, None]   # scaled x_v^T slice buffers
        rsps = [None, None]   # per-strip rowsum partials

        def e_exp(s, ci, st):
            """Energy strip [128,1024] + exp into the U half-chunk tile.

            Row-sum partials split across engines: ACT's accum_out path
            costs a ~284ns READ_ACCUMULATOR drain per strip, which made the
            scalar engine the body bottleneck (~101% busy); two of the four
            strips instead reduce the bf16 U on the DVE, which has slack."""
            ng = s * CPS + ci
            lhs = xqT[ng // 8][:, (ng % 8) * 128:(ng % 8 + 1) * 128]
            pe = psE.tile([128, 1024], F32, tag="pe", name=f"pe{s}_{ci}_{st}")
            for h in range(2):
                m0 = h * 512
                nc.tensor.matmul(pe[:, m0:m0 + 512], lhsT=lhs,
                                 rhs=(xk[st][:, m0:m0 + 512]),
                                 start=True, stop=True)
            u_dst = u_ap(ng, st // 2)[:, (st % 2) * 1024:(st % 2) * 1024 + 1024]
            if st % 2 == 0:
                nc.scalar.activation(u_dst, pe, AF.Exp, bias=nshift[:],
                                     accum_out=rsps[s % 2][:, ci, st:st + 1])
            else:
                nc.scalar.activation(u_dst, pe, AF.Exp, bias=nshift[:])
                nc.vector.tensor_reduce(out=rsps[s % 2][:, ci, st:st + 1],
                                        in_=u_dst, axis=AX.X, op=ALU.add)

        def pass1_alloc(s):
            rsps[s % 2] = rsp_p.tile([128, CPS, 4], F32, tag="rsp",
                                     name=f"rsp{s}")

        def pass1_main(s):
            pass1_alloc(s)
            for ci in range(CPS):
                for st in range(NST):
                    e_exp(s, ci, st)

        def pass1_rs(s):
            # rowsums -> 1/rs folded into scaled bf16 copy of x_v^T chunks
            xvts[s % 2] = xvsp.tile([128, CPS, CMP], BF16, tag="xvts",
                                    name=f"xvts{s}")
            for ci in range(CPS):
                ng = s * CPS + ci
                rs = small.tile([128, 1], F32, tag="rs")
                nc.vector.tensor_reduce(out=rs, in_=rsps[s % 2][:, ci, :],
                                        axis=AX.X, op=ALU.add)
                rc = small.tile([128, 1], F32, tag="rc")
                nc.vector.reciprocal(rc, rs)
                nc.vector.tensor_scalar_mul(xvts[s % 2][:, ci, :],
                                            xvt[ng][:], rc)

        def finalize_j(j):
            """Colsum renorm + transpose + subtract into x for m-chunk j."""
            cseps = small.tile([128, 1], F32, tag="cs")
            nc.vector.tensor_scalar_add(cseps, xracc[j][:, CM:CM + 1], 1e-9)
            rc2 = small.tile([128, 1], F32, tag="rc2")
            nc.vector.reciprocal(rc2, cseps)
            xrt = xrtp.tile([128, CM], BF16, tag="xrt", name=f"xrt{j}")
            nc.scalar.activation(xrt, xracc[j][:, 0:CM], AF.Copy, scale=rc2[:])
            for c in range(2):
                tp = tpp.tile([128, 128], BF16, tag="tp", name=f"tp{j}_{c}")
                nc.tensor.transpose(tp, xrt[:, c * 128:(c + 1) * 128], identb[:])
                strip, col = j // 8, (j % 8) * 128
                nc.vector.tensor_tensor(out=xs[c][strip][:, col:col + 128],
                                        in0=xs[c][strip][:, col:col + 128],
                                        in1=tp, op=ALU.subtract)

        def pass2(s):
            for j in range(NCH):
                acc = accp.tile([128, CMP], F32, tag="acc", name=f"acc{s}_{j}")
                for ci in range(CPS):
                    ng = s * CPS + ci
                    nc.tensor.matmul(acc,
                                     lhsT=(u_ap(ng, j // 16)[:, (j % 16) * 128:(j % 16) * 128 + 128]),
                                     rhs=(xvts[s % 2][:, ci, :]),
                                     start=(ci == 0), stop=(ci == CPS - 1))
                if s == 0:
                    nc.vector.tensor_scalar_add(xracc[j], acc, 0.0)
                else:
                    nc.vector.tensor_add(xracc[j], xracc[j], acc)
                if s == K - 1:
                    finalize_j(j)

        # ---- projections cascade with input strips; slice-0 energy/exp
        # rides along as soon as each xk strip lands ----
        pass1_alloc(0)
        psB = cascade_ctx.enter_context(
            tc.tile_pool(name="psB", bufs=2, space="PSUM"))
        for i in range(NST):
            # x_qT strip i (needs q strip i)
            for h in range(2):
                pt = psB.tile([CM4, 512], F32, tag="ps", name=f"pq{i}_{h}")
                nc.tensor.matmul(pt, lhsT=(wqT[:]),
                                 rhs=(qs[i][:, h * 512:(h + 1) * 512]),
                                 start=True, stop=True)
                nc.vector.tensor_scalar_add(xqT[i][:, h * 512:(h + 1) * 512],
                                            pt, 0.0)
            # x_vT chunks of strip i
            for nl in range(8):
                ni = i * 8 + nl
                pt = psB.tile([128, CMP], F32, tag="ps", name=f"pv{ni}")
                nc.tensor.matmul(pt, lhsT=(qs[i][:, nl * 128:(nl + 1) * 128]),
                                 rhs=(wvT[:]), start=True, stop=True)
                nc.vector.tensor_add(xvt[ni][:], pt, bvb)
            # x_k strip i (needs both x chunks of strip i)
            for h in range(2):
                pt = psB.tile([CM4, 512], F32, tag="ps", name=f"pk{i}_{h}")
                for k in range(2):
                    nc.tensor.matmul(pt, lhsT=(wkT[k][:]),
                                     rhs=(xs[k][i][:, h * 512:(h + 1) * 512]),
                                     start=(k == 0), stop=(k == 1))
                nc.vector.tensor_scalar_add(xk[i][:, h * 512:(h + 1) * 512],
                                            pt, 0.0)
            # slice-0 energy/exp for m-strip i, all 4 chunks
            for ci in range(CPS):
                e_exp(0, ci, i)

        cascade_ctx.close()
        accp = psE_ctx.enter_context(tc.tile_pool(name="accp", bufs=4, space="PSUM"))
        tpp = None
        xrtp = None

        # CC ring warm-keepers: a cold collective costs ~30us, a warm one
        # ~7us. Tiny AllGathers tied to slice progress keep the CC cores
        # warm through the body so the BN-stats gathers at the end run at
        # the warm rate (they also absorb the inter-core launch skew).
        def cc_warm(s):
            cin_d = warmp.tile([1, 4], F32, tag=f"wi{s}", name=f"cin_d{s}")
            cout_d = warmp.tile([NCORES, 1, 4], F32, tag=f"wo{s}",
                                name=f"cout_d{s}")
            nc.sync.dma_start(cin_d, xracc[0][0:1, 0:4])
            nc.gpsimd.collective_compute(
                "AllGather", ALU.bypass,
                replica_groups=[list(range(NCORES))],
                ins=[cin_d.opt()], outs=[cout_d.opt()])

        # ---- sliced, software-pipelined attention ----
        pass1_rs(0)
        for s in range(K):
            if s + 1 < K:
                pass1_main(s + 1)
            if s == K - 1:
                # pass1 fully emitted: release psE's 4 banks (and the old
                # accp) and open a fresh accp + the transpose pool for the
                # final slice's pass2 + finalize_j
                psE_ctx.close()
                accp = actx.enter_context(
                    tc.tile_pool(name="accp2", bufs=4, space="PSUM"))
                tpp = actx.enter_context(
                    tc.tile_pool(name="tpp", bufs=2, space="PSUM"))
                xrtp = actx.enter_context(tc.tile_pool(name="xrtp", bufs=3))
            pass2(s)
            if s in (1, 3, 5, 6):
                cc_warm(s)
            if s + 1 < K:
                pass1_rs(s + 1)
        actx.close()
        warm_ctx.close()

        # ---- t = WtT^T @ d;  BN stats; AllGather; relu(a*t+b) ----
        # A single stats AllGather: the warm-keepers already absorbed the
        # launch skew and keep the ring warm, so it runs at the ~7us warm
        # rate and a split would only serialize two ops.
        stats = consts.tile([128, 4], F32)
        gstats = consts.tile([128, 4], F32)
        tst = [pbig.tile([128, 1024], F32, tag=f"uh{p}", name=f"tst{p}")
               for p in range(8)]

        def ts_ap(oc, lo, hi):
            p = (oc * N + lo) // 1024
            return tst[p][:, lo % 1024:(lo % 1024) + (hi - lo)]
        tsums = consts.tile([128, 2, NMG], F32)
        qsums = consts.tile([128, 2, NMG], F32)
        g8 = consts.tile([128, NCORES, 4], F32)
        with tc.tile_pool(name="psT", bufs=3, space="PSUM") as psT, \
             tc.tile_pool(name="sqp", bufs=2) as sqp:
            for mi in range(NMG):
                for oc in range(2):
                    pt = psT.tile([128, 512], F32, tag="t", name=f"pt{oc}_{mi}")
                    for kc in range(2):
                        nc.tensor.matmul(pt,
                                         lhsT=(wtT[kc][:, oc * 128:(oc + 1) * 128]),
                                         rhs=(xs[kc][mi // 2][:, (mi % 2) * 512:(mi % 2) * 512 + 512]),
                                         start=(kc == 0), stop=(kc == 1))
                    # DVE copy-with-accum: ts strip + per-strip channel sum
                    nc.vector.tensor_scalar(
                        out=ts_ap(oc, mi * 512, (mi + 1) * 512), in0=pt,
                        scalar1=1.0, scalar2=0.0, op0=ALU.mult, op1=ALU.add,
                        accum_out=tsums[:, oc, mi:mi + 1])
                    # ACT square-with-accum from PSUM: per-strip sumsq
                    sq = sqp.tile([128, 512], F32, tag="sq", name=f"sq{oc}_{mi}")
                    nc.scalar.activation(sq, pt, AF.Square,
                                         accum_out=qsums[:, oc, mi:mi + 1])
            for oc in range(2):
                nc.vector.tensor_reduce(out=stats[:, oc:oc + 1],
                                        in_=tsums[:, oc, :], axis=AX.X, op=ALU.add)
                nc.vector.tensor_reduce(out=stats[:, 2 + oc:3 + oc],
                                        in_=qsums[:, oc, :], axis=AX.X, op=ALU.add)
        with tc.tile_pool(name="dramp", bufs=1, space="DRAM") as dramp:
            cin = dramp.tile([128, 4], F32)
            cout = dramp.tile([NCORES, 128, 4], F32)
            nc.sync.dma_start(cin, stats)
            nc.gpsimd.collective_compute(
                "AllGather", ALU.bypass,
                replica_groups=[list(range(NCORES))],
                ins=[cin.opt()], outs=[cout.opt()])
            nc.sync.dma_start(g8, cout[:].rearrange("r p c -> p r c"))
        nc.vector.tensor_reduce(out=gstats[:],
                                in_=g8.rearrange("p r c -> p c r"),
                                axis=AX.X, op=ALU.add)

        invn = 1.0 / float(B * N)
        mean = small.tile([128, 2], F32, tag="bn")
        ex2 = small.tile([128, 2], F32, tag="bn")
        var = small.tile([128, 2], F32, tag="bn")
        sd = small.tile([128, 2], F32, tag="bn")
        rstd = small.tile([128, 2], F32, tag="bn2")
        a_ = small.tile([128, 2], F32, tag="bn2")
        amean = small.tile([128, 2], F32, tag="bn2")
        b_ = small.tile([128, 2], F32, tag="bn2")
        nc.vector.tensor_scalar_mul(mean, gstats[:, 0:2], invn)
        nc.vector.tensor_scalar_mul(ex2, gstats[:, 2:4], invn)
        nc.vector.tensor_mul(var, mean, mean)
        nc.vector.tensor_sub(var, ex2, var)
        nc.vector.tensor_scalar_add(var, var, BN_EPS)
        nc.scalar.activation(sd, var, AF.Sqrt)
        nc.vector.reciprocal(rstd, sd)
        nc.vector.tensor_mul(a_, gac, rstd)
        nc.vector.tensor_mul(amean, a_, mean)
        nc.vector.tensor_sub(b_, bec, amean)

        # relu + store, pipelined per 1024-strip (reuse xs strip space);
        # strips split ACT/DVE so the two engines drain the tail in parallel
        for k, (oc, i) in enumerate((oc, i) for oc in range(2)
                                    for i in range(NST)):
            o = pbig.tile([128, 1024], F32, tag=f"x{oc}_{i}", name=f"o{oc}_{i}")
            src = ts_ap(oc, i * 1024, (i + 1) * 1024)
            if k % 3 == 2:
                nc.vector.tensor_scalar(out=o, in0=src,
                                        scalar1=a_[:, oc:oc + 1],
                                        scalar2=b_[:, oc:oc + 1],
                                        op0=ALU.mult, op1=ALU.add)
                nc.vector.tensor_scalar_max(o, o, 0.0)
            else:
                nc.scalar.activation(o, src, AF.Relu,
                                     bias=b_[:, oc:oc + 1],
                                     scale=a_[:, oc:oc + 1])
            q = nc.sync if k % 2 == 0 else nc.gpsimd
            q.dma_start(dout[oc * 128:(oc + 1) * 128,
                             i * 1024:(i + 1) * 1024], o)


_CACHE = {}


def _get_nc():
    if "nc" not in _CACHE:
        _CACHE["nc"] = build_nc()
    return _CACHE["nc"]


def _make_in_maps(inputs):
    f = lambda a: np.ascontiguousarray(np.asarray(a, dtype=np.float32))
    q = f(inputs["q"])
    x = f(inputs["x"])
    shared = {
        "wqT": f(np.asarray(inputs["Wq"]).T),
        "wkT": f(np.asarray(inputs["Wk"]).T),
        "wvT": f(np.concatenate([np.asarray(inputs["Wv"]).T, np.zeros((DX, 2), np.float32)], axis=1)),
        "wtT": f(np.asarray(inputs["Wt"]).T),
        "bvb": f(np.concatenate([np.broadcast_to(np.asarray(inputs["bv"]), (128, CM)), np.ones((128, 1), np.float32), np.zeros((128, 1), np.float32)], axis=1)),
        "gac": f(np.asarray(inputs["gamma"]).reshape(2, 128).T),
        "bec": f(np.asarray(inputs["beta"]).reshape(2, 128).T),
    }
    return [{"q": q[b], "x": x[b], **shared} for b in range(NCORES)]


def run(inputs, trace=False):
    """Run on the 8 NeuronCores; returns (out [8,256,4096], exec_time_ns|None)."""
    from concourse.bass_utils import run_bass_kernel_spmd

    nc = _get_nc()
    in_maps = _make_in_maps(inputs)
    res = run_bass_kernel_spmd(nc, in_maps, core_ids=list(range(NCORES)),
                               trace=trace)
    out = np.stack([res.results[b]["out"] for b in range(NCORES)], axis=0)
    return out, res.exec_time_ns


def kernel(**inputs) -> np.ndarray:
    out, _ = run(inputs)
    return out

